# revision 1
# baseline (speedup 1.0000x reference)
"""Trainium2 Bass kernel for nn_BilinearAttnPool (B=32, C=2048, H=24, W=12, M=8).

Math notes (why this is exact enough):
  reference: attn = relu(BN(conv1x1(f)))  (attn >= 0)
             x = clip(f * attn, min=1e-6) ** 3 ; pooled = mean_hw(x) ** (1/3)
  Since attn >= 0:  clip(f*attn, eps) = attn * relu(f)  except where f*attn <= eps,
  where the reference contributes eps^3 = 1e-18 (utterly negligible vs O(0.1) sums,
  and no (m,c) pair is fully dead for these inputs).  Therefore
      pooled(b,m,c)^3 ∝ sum_hw attn(b,m,hw)^3 * relu(f(b,c,hw))^3        -- a matmul!
  relu(f)^3 = f^2 * max(f, 0)  (one ACT Square + one DVE max + one DVE mul).
  The 1/HW mean factor is a global scale that cancels in the final L2 normalize,
  as does any global constant through the ^(1/3) and sign-sqrt (monotone powers).
  pooled >= 0 always => sign-sqrt == sqrt;  z = s^(1/6) = exp(log(s)/6).
  L2 norm >> 1e-12 here so max(norm, eps) == norm.

Layout notes:
  features are loaded with partition p holding channels c in [16p, 16p+16)
  (fully contiguous 18KB per partition per sample -> line-rate DMA).  The
  pooled PSUM columns come out c-permuted (col = (c%16)*128 + c//16); the
  final output DMA un-permutes with a strided SBUF-side access pattern.
  relu(f)^3 is written in a "tails gathered" layout: per c-group i the 288
  hw values are split 256 + 32, all 16 groups' 32-tails packed together, so
  the whole per-sample tile is transposed by ONE hw-aligned DMA-xbar
  transpose ([128, 4608] -> [128, 36, 128], all blocks 128-aligned, no pad).
  The hw contraction then runs as K=128,128,32 matmuls; the K=32 tail mms
  use tile_position row offsets (32*(i%4)) to hit the packed tail rows.

Sharding: pure data parallel, batch 32 -> 8 cores x 4 samples.
"""

import numpy as np
import ml_dtypes

B, C, H, W, M = 32, 2048, 24, 12, 8
NCORES = 8
BL = B // NCORES          # 4 samples per core
HW = H * W                # 288
P = 128
CI = C // P               # 16 channels per partition
HWP = 384                 # a3 pad (transpose alignment only)
NH = 3
BN_EPS = 1e-3

_CACHE = {}


def _build_program():
    import concourse.bass as bass
    import concourse.tile as tile
    import concourse.mybir as mybir
    from concourse import bacc

    # Pin every ACT function to the one table set that contains all of
    # Square/Relu/Ln/Exp/Copy, so the whole kernel does a single
    # ACT_TABLE_LOAD instead of ping-ponging between sets (~1.3us each).
    import concourse.hw_specs as hw_specs
    import concourse.bacc as bacc_mod
    _orig_tables = bacc_mod.get_activation_tables

    def _pinned_tables(arch):
        tabs = dict(_orig_tables(arch))
        if "natural_log_exp_and_others" in tabs:
            for k in tabs:
                if k != "natural_log_exp_and_others":
                    tabs[k] = set()
        return tabs

    bacc_mod.get_activation_tables = _pinned_tables
    try:
        nc = _build_inner(bacc_mod, tile, mybir)
    finally:
        bacc_mod.get_activation_tables = _orig_tables
    return nc


def _build_inner(bacc, tile, mybir):
    dt = mybir.dt
    AF = mybir.ActivationFunctionType
    ALU = mybir.AluOpType

    nc = bacc.Bacc("TRN2", target_bir_lowering=False, debug=False,
                   num_devices=NCORES)

    feats = nc.declare_dram_parameter("feats", [BL, C, HW], dt.float32, isOutput=False)
    w2t_d = nc.declare_dram_parameter("w2t", [P, CI * M], dt.bfloat16, isOutput=False)
    dvec_d = nc.declare_dram_parameter("dvec", [P, 1], dt.float32, isOutput=False)
    gmat_d = nc.declare_dram_parameter("gmat", [P, BL], dt.float32, isOutput=False)
    gmat2_d = nc.declare_dram_parameter("gmat2", [BL, P], dt.float32, isOutput=False)
    ident_d = nc.declare_dram_parameter("ident", [P, 32], dt.bfloat16, isOutput=False)
    out_d = nc.declare_dram_parameter("out", [BL, M, C], dt.float32, isOutput=True)

    with tile.TileContext(nc) as tc:
        with (
            tc.tile_pool(name="const", bufs=1) as cpool,
            tc.tile_pool(name="persist", bufs=1) as perst,
            tc.tile_pool(name="fpool", bufs=BL) as fpool,
            tc.tile_pool(name="spool", bufs=2) as spool,
            tc.tile_pool(name="rpool", bufs=2) as rpool,
            tc.tile_pool(name="ppool", bufs=3) as ppool,
            tc.tile_pool(name="tpool", bufs=3) as tpool,
            tc.tile_pool(name="psa", bufs=1, space="PSUM") as psa_pool,
            tc.tile_pool(name="psp", bufs=1, space="PSUM") as psp_pool,
            tc.tile_pool(name="psn", bufs=1, space="PSUM") as psn_pool,
        ):
            w2t = cpool.tile([P, CI * M], dt.bfloat16)
            dvec = cpool.tile([P, 1], dt.float32)
            gmat = cpool.tile([P, BL], dt.float32)
            gmat2 = cpool.tile([BL, P], dt.float32)
            ident = cpool.tile([P, 32], dt.bfloat16)
            nc.gpsimd.dma_start(ident[:], ident_d.ap())
            nc.sync.dma_start(w2t[:], w2t_d.ap())
            nc.sync.dma_start(dvec[:], dvec_d.ap())
            nc.sync.dma_start(gmat[:], gmat_d.ap())
            nc.sync.dma_start(gmat2[:], gmat2_d.ap())

            attn = perst.tile([P, HW], dt.bfloat16)
            sqt = perst.tile([P, HW], dt.bfloat16)
            a3 = perst.tile([P, HWP], dt.bfloat16)
            a3t = [perst.tile([P, NH, 32], dt.bfloat16, name=f"a3t{b}",
                              tag=f"a3t{b}") for b in range(BL)]

            psA = psa_pool.tile([P, HW], dt.float32)
            psP = psp_pool.tile([P, C], dt.float32)
            psT = psn_pool.tile([P, NH, 32], dt.bfloat16, tag="pst")
            psN = psn_pool.tile([BL, 1], dt.float32, tag="psn4")
            psB = psn_pool.tile([P, 1], dt.float32, tag="psnb")

            nc.vector.memset(a3[:], 0.0)
            nc.vector.memset(psP[:], 1.0)    # keep log() finite on unused rows

            # all feature loads issued up-front: fp32 HBM -> bf16 SBUF,
            # one fully-contiguous 18KB run per partition per sample
            fbs = []
            for b in range(BL):
                fb = fpool.tile([P, CI, HW], dt.bfloat16)
                nc.gpsimd.dma_start(
                    fb[:], feats.ap()[b].rearrange("(p i) hw -> p i hw", p=P))
                fbs.append(fb)

            # ---- software-pipelined emission: per-engine streams are
            # in-order, so trace order is scheduled order per engine ----
            pbs = [None] * BL
            ptbs = [None] * BL

            def elementwise(b):
                # s = f^2 (ACT), r = relu(f) (DVE), p = s*r = relu(f)^3 (DVE)
                # p layout: cols [0:512) hw tails (i, t); then 512+256i+j
                fb = fbs[b]
                s = spool.tile([P, CI, HW], dt.bfloat16)
                r = rpool.tile([P, CI, HW], dt.bfloat16)
                pb = ppool.tile([P, CI * HW], dt.bfloat16)
                pbB = pb[:, 0:CI * 32].rearrange("p (i j) -> p i j", i=CI)
                pbA = pb[:, CI * 32:CI * HW].rearrange("p (i j) -> p i j", i=CI)
                nc.scalar.activation(s[:], fb[:], AF.Square)
                nc.vector.tensor_scalar_max(r[:], fb[:], 0.0)
                nc.vector.tensor_mul(pbA[:], s[:, :, 0:256], r[:, :, 0:256])
                nc.vector.tensor_mul(pbB[:], s[:, :, 256:HW], r[:, :, 256:HW])
                pbs[b] = pb
                # big serial-ring transpose (blocks: 0-3 tails, 4+i = h0
                # of c-group i, 20+i = h1 of c-group i)
                ptb = tpool.tile([P, 2 * CI + 4, P], dt.bfloat16,
                                 name="ptb", tag="ptb")
                nc.sync.dma_start_transpose(ptb[:], pb[:])
                ptbs[b] = ptb

            def conv(b):
                for i in range(CI):
                    nc.tensor.matmul(
                        psA[32 * b:32 * b + M, :],
                        w2t[:, M * i:M * (i + 1)],
                        fbs[b][:, i, :],
                        start=(i == 0), stop=(i == CI - 1),
                        tile_position=(0, 32 * b),
                        skip_group_check=True,
                    )

            def attn_cube(b):
                # attn = relu(conv*g + d) on DVE (ACT is busy with squares);
                # a3 = attn^3 with the 32-wide hw tail replicated into the
                # pad columns (the K=32 tail matmuls slice it at 32j)
                rs = slice(32 * b, 32 * b + M)
                nc.scalar.activation(attn[rs, :], psA[rs, :], AF.Relu,
                                     bias=dvec[rs, :])
                nc.vector.tensor_mul(sqt[rs, :], attn[rs, :], attn[rs, :])
                nc.vector.tensor_mul(a3[rs, 0:HW], sqt[rs, :], attn[rs, :])
                for k in range(1, 4):
                    nc.vector.tensor_copy(a3[rs, 256 + 32 * k:288 + 32 * k],
                                          a3[rs, 256:288])
                # a3 transpose on PE (the DMA xbar would serialize against
                # the in-flight feature loads); copy PSUM->SBUF on DVE
                for k in range(NH):
                    nc.tensor.transpose(
                        psT[:, k, :],
                        a3[32 * b:32 * b + 32, P * k:P * (k + 1)],
                        ident[32 * b:32 * b + 32, :],
                        tile_position=(32 * b, 0))
                nc.vector.tensor_copy(a3t[b][:], psT[:])

            def pooled(b):
                rs = slice(32 * b, 32 * b + M)
                ptb = ptbs[b]
                for i in range(CI):
                    j = i % 4
                    cs = slice(P * i, P * (i + 1))
                    nc.tensor.matmul(
                        psP[rs, cs],
                        a3t[b][32 * j:32 * j + 32, 2, 0:M],
                        ptb[32 * j:32 * j + 32, i // 4, :],
                        start=True, stop=False,
                        tile_position=(32 * j, 32 * b),
                        skip_group_check=True)
                    nc.tensor.matmul(
                        psP[rs, cs], a3t[b][:, 0, 0:M], ptb[:, 4 + 2 * i, :],
                        start=False, stop=False, tile_position=(0, 32 * b),
                        skip_group_check=True)
                    nc.tensor.matmul(
                        psP[rs, cs], a3t[b][:, 1, 0:M], ptb[:, 5 + 2 * i, :],
                        start=False, stop=True, tile_position=(0, 32 * b),
                        skip_group_check=True)

            for b in range(BL):
                elementwise(b)
                conv(b)
                attn_cube(b)
                pooled(b)

            # post: z = s^(1/6);  n = sum_{m,c} z^2;  out = z / sqrt(n)
            lns = perst.tile([P, C], dt.float32)
            z = perst.tile([P, C], dt.float32)
            fm = perst.tile([P, C], dt.float32)
            part = perst.tile([P, 1], dt.float32)
            n4 = perst.tile([BL, 1], dt.float32)
            lnn = perst.tile([P, 1], dt.float32)
            rn = perst.tile([P, 1], dt.float32)

            nc.scalar.activation(lns[:], psP[:], AF.Ln)
            nc.scalar.activation(z[:], lns[:], AF.Exp, scale=1.0 / 6.0)
            # z^2 and its row-sum in one fused ACT op (lns is dead scratch)
            nc.scalar.activation(lns[:], z[:], AF.Square, accum_out=part[:])
            nc.tensor.matmul(psN[:], gmat[:], part[:])          # [4,1] group sums
            nc.scalar.copy(n4[:], psN[:])
            nc.tensor.matmul(psB[:], gmat2[:], n4[:])           # broadcast [128,1]
            nc.scalar.activation(lnn[:], psB[:], AF.Ln)
            nc.scalar.activation(rn[:], lnn[:], AF.Exp, scale=-0.5)
            nc.vector.tensor_scalar_mul(fm[:], z[:], rn[:])

            for b in range(BL):
                nc.gpsimd.dma_start(out_d.ap()[b], fm[32 * b:32 * b + M, :])

    nc.compile()
    return nc


def _host_prep(conv_w, bn_scale, bn_bias, bn_mean, bn_var):
    g = (bn_scale / np.sqrt(bn_var + BN_EPS)).astype(np.float32)
    d = (bn_bias - bn_mean * g).astype(np.float32)
    w2 = (conv_w.astype(np.float32) * g[:, None])          # [M, C]
    # lhsT layout: [p, i*8+m] = w2[m, 16p+i]
    w2t = np.ascontiguousarray(
        w2.T.reshape(P, CI, M)).astype(ml_dtypes.bfloat16).reshape(P, CI * M)
    dvec = np.zeros((P, 1), np.float32)
    gmat = np.zeros((P, BL), np.float32)
    for b in range(BL):
        dvec[32 * b:32 * b + M, 0] = d
        gmat[32 * b:32 * b + M, b] = 1.0
    gmat2 = np.ascontiguousarray(gmat.T)
    ident = np.tile(np.eye(32, dtype=np.float32), (4, 1)).astype(ml_dtypes.bfloat16)
    return w2t, dvec, gmat, gmat2, ident


def kernel(features, conv_w, bn_scale, bn_bias, bn_mean, bn_var, **_kw):
    from concourse.bass_utils import run_bass_kernel_spmd

    if "nc" not in _CACHE:
        _CACHE["nc"] = _build_program()
    nc = _CACHE["nc"]

    feats = np.ascontiguousarray(np.asarray(features, np.float32)).reshape(B, C, HW)
    w2t, dvec, gmat, gmat2, ident = _host_prep(
        np.asarray(conv_w, np.float32), np.asarray(bn_scale, np.float32),
        np.asarray(bn_bias, np.float32), np.asarray(bn_mean, np.float32),
        np.asarray(bn_var, np.float32))

    in_maps = []
    for i in range(NCORES):
        in_maps.append({
            "feats": np.ascontiguousarray(feats[BL * i:BL * (i + 1)]),
            "w2t": w2t, "dvec": dvec, "gmat": gmat, "gmat2": gmat2,
            "ident": ident,
        })

    res = run_bass_kernel_spmd(nc, in_maps, core_ids=list(range(NCORES)),
                               **_CACHE.get("run_kwargs", {}))
    _CACHE["last_results"] = res
    # device col k = (c%16)*128 + c//16  ->  un-permute on host
    inv = ((np.arange(C) % CI) * P + np.arange(C) // CI)
    out = np.concatenate(
        [res.results[i]["out"][:, :, inv].reshape(BL, M * C)
         for i in range(NCORES)], axis=0)
    return np.ascontiguousarray(out.reshape(B, M * C, 1, 1).astype(np.float32))

